# revision 2
# baseline (speedup 1.0000x reference)
"""FourDirGradientConv + 1x1 compress + BatchNorm, Trainium2 Bass kernel (v2).

Math: feat = concat_g(shift_g(x) - x), y = W @ feat, out = BN(y) * gamma + beta
with shifts g in {(-1,+1), (-1,-1), (+1,+1), (+1,-1)} (zero-padded).

v2 rewrite: output tiles are [128 partitions = 32 rows x 4 channels, 512 cols].
x lives in SBUF as [128 part = (row%4, ch), 128 blocks of 4 rows, 514 cols]
(zero pad cols). A column shift dj is a free-dim offset on the rhs; a row
shift di is encoded in the lhsT (which image row block each K row feeds).
Per 32-row tile: 8 interior 4-row blocks x 3 column shifts + 4 edge matmuls
= 28 matmuls (vs 40 in v1's M=16 scheme), all M=128, N=512, K=128.

Sharding: data-parallel over batch, core b <-> sample b. BN batch stats are a
[4,2] AllReduce across the 8 cores.
"""

import os
import numpy as np

import concourse.bass as bass
import concourse.tile as tile
import concourse.mybir as mybir
from concourse.bass_utils import run_bass_kernel_spmd

# problem constants (hardcoded per harness contract)
B, C, H, W = 8, 32, 512, 512
BN_EPS = 1e-5
N_CORES = 8

WP = W + 2  # padded row width in SBUF
NB = H // 4  # 128 blocks of 4 rows
TILES = H // 32  # 16 output tiles of 32 rows
CHUNK = 8  # blocks per load DMA
NCHUNK = NB // CHUNK  # 16 load DMAs

F32 = mybir.dt.float32
_DT_MAP = {
    "f32": mybir.dt.float32,
    "f16": mybir.dt.float16,
    "bf16": mybir.dt.bfloat16,
}
MM_DT = _DT_MAP[os.environ.get("BASSK_DT", "f16")]

# (j, dj) order for the lhsT table; j = block position within tile (-1..8),
# dj = column shift. Edge blocks only carry the off-row taps.
W_ORDER = []
for _j in range(-1, 9):
    for _dj in (-1, 0, 1):
        if _j in (-1, 8) and _dj == 0:
            continue
        W_ORDER.append((_j, _dj))
W_IDX = {jd: i for i, jd in enumerate(W_ORDER)}
NW = len(W_ORDER)  # 28


def _split_multiwait(nc, max_waits=1):
    """Walrus here rejects >1 sync wait per instruction (tail Drain carries
    several); hoist extras onto same-engine NOPs placed just before."""
    for f in nc.m.functions:
        for b in f.blocks:
            insts = list(b.instructions)
            out = []
            changed = False
            for inst in insts:
                si = inst.sync_info
                if si is not None and len(si.on_wait) > max_waits:
                    waits = list(si.on_wait)
                    keep = waits[-max_waits:]
                    for k, wt in enumerate(waits[:-max_waits]):
                        out.append(
                            mybir.InstNoOp(
                                name=f"{inst.name}-waitsplit-{k}",
                                engine=inst.engine,
                                sync_info=mybir.SyncInfo(on_wait=[wt], on_update=[]),
                            )
                        )
                    inst.sync_info = mybir.SyncInfo(
                        on_wait=keep, on_update=list(si.on_update)
                    )
                    changed = True
                out.append(inst)
            if changed:
                b.instructions = out


def build_module():
    nc = bass.Bass(num_devices=N_CORES)

    # xq: host-pretransposed input, xq[q, ch, b, c] = x[ch, 4b+q, c]
    xb = nc.declare_dram_parameter("xq", [4, C, NB, W], F32, isOutput=False)
    wst = nc.declare_dram_parameter("wst", [NW, 128, 128], F32, isOutput=False)
    sel = nc.declare_dram_parameter("sel", [128, 4], F32, isOutput=False)
    selbc = nc.declare_dram_parameter("selbc", [4, 128], F32, isOutput=False)
    gamma = nc.declare_dram_parameter("gamma", [4, 1], F32, isOutput=False)
    beta = nc.declare_dram_parameter("beta", [4, 1], F32, isOutput=False)
    # y laid out as [rr, o, t, c] = out[o, 32t+rr, c]; host transposes back
    y = nc.declare_dram_parameter("y", [32, 4, TILES, W], F32, isOutput=True)

    with tile.TileContext(nc, num_cores=N_CORES) as tc:
        with (
            tc.tile_pool(name="xp", bufs=1) as xp,
            tc.tile_pool(name="const", bufs=1) as constp,
            tc.tile_pool(name="ysb", bufs=1) as ysbp,
            tc.tile_pool(name="stats", bufs=1) as statsp,
            tc.tile_pool(name="small", bufs=1) as smallp,
            tc.tile_pool(name="ps", bufs=3, space="PSUM") as psp,
            tc.tile_pool(name="pss", bufs=1, space="PSUM") as pssp,
            tc.tile_pool(name="dram", bufs=1, space="DRAM") as dramp,
        ):
            # constants; wst is cast to MM_DT by the gpsimd (SWDGE) DMA
            w_sb = constp.tile([128, NW, 128], MM_DT)
            if MM_DT is F32:
                nc.sync.dma_start(out=w_sb[:], in_=wst.transpose([1, 0, 2]))
            else:
                nc.gpsimd.dma_start(out=w_sb[:], in_=wst.transpose([1, 0, 2]))
            sel_sb = constp.tile([128, 4], F32)
            nc.sync.dma_start(out=sel_sb[:], in_=sel[:])
            selbc_sb = constp.tile([4, 128], F32)
            nc.sync.dma_start(out=selbc_sb[:], in_=selbc[:])
            gamma_sb = constp.tile([4, 1], F32)
            nc.sync.dma_start(out=gamma_sb[:], in_=gamma[:])
            beta_sb = constp.tile([4, 1], F32)
            nc.sync.dma_start(out=beta_sb[:], in_=beta[:])
            eps_sb = constp.tile([4, 1], F32)
            nc.gpsimd.memset(eps_sb[:], BN_EPS)

            # resident x: partition (q=row%4, ch), free (block, padded col)
            x_t = xp.tile([128, NB, WP], MM_DT)
            nc.gpsimd.memset(x_t[:, :, 0:1], 0.0)
            nc.gpsimd.memset(x_t[:, :, WP - 1 : WP], 0.0)

            for k in range(NCHUNK):
                b0 = CHUNK * k
                dst = x_t[:, b0 : b0 + CHUNK, 1 : W + 1]
                if MM_DT is F32:
                    nc.sync.dma_start(out=dst, in_=xb[:, :, b0 : b0 + CHUNK, :])
                else:
                    nc.gpsimd.dma_start(out=dst, in_=xb[:, :, b0 : b0 + CHUNK, :])

            # y storage: partition (rr, o) = 4*rr + o, free (tile, col)
            y_sb = ysbp.tile([128, TILES, W], F32)
            stats = statsp.tile([128, TILES, 6], F32)

            for t in range(TILES):
                ps = psp.tile([128, W], F32)
                emitted = []
                for j in range(-1, 9):
                    b = 8 * t + j
                    if b < 0 or b >= NB:
                        continue
                    for dj in (-1, 0, 1):
                        if j in (-1, 8) and dj == 0:
                            continue
                        emitted.append((W_IDX[(j, dj)], b, dj))
                for n, (idx, b, dj) in enumerate(emitted):
                    nc.tensor.matmul(
                        out=ps[:],
                        lhsT=w_sb[:, idx, :],
                        rhs=x_t[:, b, 1 + dj : 1 + dj + W],
                        start=(n == 0),
                        stop=(n == len(emitted) - 1),
                    )
                nc.scalar.copy(out=y_sb[:, t, :], in_=ps[:])
                nc.vector.bn_stats(out=stats[:, t, :], in_=ps[:])

            # ---- BN stats: combine partials over partitions ----
            mv = smallp.tile([128, 2], F32)
            nc.vector.bn_aggr(out=mv[:], in_=stats[:])

            # S12: col0 = mean_p, col1 = mean_p^2 + var_p (per-partition E[y^2])
            s12 = smallp.tile([128, 2], F32)
            nc.vector.tensor_copy(out=s12[:, 0:1], in_=mv[:, 0:1])
            nc.vector.tensor_tensor(
                out=s12[:, 1:2], in0=mv[:, 0:1], in1=mv[:, 0:1],
                op=mybir.AluOpType.mult,
            )
            nc.vector.tensor_tensor(
                out=s12[:, 1:2], in0=s12[:, 1:2], in1=mv[:, 1:2],
                op=mybir.AluOpType.add,
            )

            # combine over partitions: out[o, t] = sum_p sel[p,o] * s12[p,t]
            comb_ps = pssp.tile([4, 2], F32)
            nc.tensor.matmul(
                out=comb_ps[:], lhsT=sel_sb[:], rhs=s12[:], start=True, stop=True
            )
            comb = smallp.tile([4, 2], F32)
            nc.scalar.copy(out=comb[:], in_=comb_ps[:])

            # ---- AllReduce across cores ----
            cc_in = dramp.tile([4, 2], F32)
            cc_out = dramp.tile([4, 2], F32)
            nc.sync.dma_start(out=cc_in[:], in_=comb[:])
            nc.gpsimd.collective_compute(
                "AllReduce",
                mybir.AluOpType.add,
                replica_groups=[list(range(N_CORES))],
                ins=[cc_in.opt()],
                outs=[cc_out.opt()],
            )
            arin = smallp.tile([4, 2], F32)
            nc.sync.dma_start(out=arin[:], in_=cc_out[:])

            # ---- scale/bias math on [4,1] ----
            # 32 partitions per channel x 8 cores = 256 partials
            mean = smallp.tile([4, 1], F32)
            nc.scalar.mul(out=mean[:], in_=arin[:, 0:1], mul=1.0 / 256.0)
            var = smallp.tile([4, 1], F32)
            nc.scalar.mul(out=var[:], in_=arin[:, 1:2], mul=1.0 / 256.0)
            msq = smallp.tile([4, 1], F32)
            nc.vector.tensor_tensor(
                out=msq[:], in0=mean[:], in1=mean[:], op=mybir.AluOpType.mult
            )
            nc.vector.tensor_tensor(
                out=var[:], in0=var[:], in1=msq[:], op=mybir.AluOpType.subtract
            )
            sd = smallp.tile([4, 1], F32)
            nc.scalar.activation(
                out=sd[:], in_=var[:], func=mybir.ActivationFunctionType.Sqrt,
                bias=eps_sb[:], scale=1.0,
            )
            rstd = smallp.tile([4, 1], F32)
            nc.vector.reciprocal(out=rstd[:], in_=sd[:])
            scbi = smallp.tile([4, 2], F32)
            nc.vector.tensor_tensor(
                out=scbi[:, 0:1], in0=gamma_sb[:], in1=rstd[:],
                op=mybir.AluOpType.mult,
            )
            tmp = smallp.tile([4, 1], F32)
            nc.vector.tensor_tensor(
                out=tmp[:], in0=mean[:], in1=scbi[:, 0:1], op=mybir.AluOpType.mult
            )
            nc.vector.tensor_tensor(
                out=scbi[:, 1:2], in0=beta_sb[:], in1=tmp[:],
                op=mybir.AluOpType.subtract,
            )
            # broadcast to [128, 2]: out[p, t] = scbi[p % 4, t]
            bc_ps = pssp.tile([128, 2], F32)
            nc.tensor.matmul(
                out=bc_ps[:], lhsT=selbc_sb[:], rhs=scbi[:], start=True, stop=True
            )
            scv = smallp.tile([128, 2], F32)
            nc.scalar.copy(out=scv[:], in_=bc_ps[:])

            # ---- affine + store out (2 halves for DVE/DMA overlap) ----
            HT = TILES // 2
            for h in range(2):
                sl = slice(h * HT, (h + 1) * HT)
                nc.vector.tensor_scalar(
                    out=y_sb[:, sl, :],
                    in0=y_sb[:, sl, :],
                    scalar1=scv[:, 0:1],
                    scalar2=scv[:, 1:2],
                    op0=mybir.AluOpType.mult,
                    op1=mybir.AluOpType.add,
                )
                nc.sync.dma_start(out=y[:, :, sl, :], in_=y_sb[:, sl, :])

    _split_multiwait(nc)
    return nc


def _host_constants(w_compress):
    # taps: (di, dj, weight[4,32])
    wg = [w_compress[:, 32 * g : 32 * g + 32] for g in range(4)]
    taps = [
        (-1, +1, wg[0]),  # ne
        (-1, -1, wg[1]),  # nw
        (+1, +1, wg[2]),  # se
        (+1, -1, wg[3]),  # sw
        (0, 0, -(wg[0] + wg[1] + wg[2] + wg[3])),  # center
    ]
    wst = np.zeros((NW, 128, 128), dtype=np.float32)
    for (j, dj), idx in W_IDX.items():
        for di, djg, wm in taps:
            if djg != dj:
                continue
            for q in range(4):
                rr = 4 * j + q - di
                if 0 <= rr < 32:
                    wst[idx, 32 * q : 32 * q + 32, 4 * rr : 4 * rr + 4] = wm.T

    sel = np.zeros((128, 4), dtype=np.float32)
    for prt in range(128):
        sel[prt, prt % 4] = 1.0
    selbc = np.zeros((4, 128), dtype=np.float32)
    for prt in range(128):
        selbc[prt % 4, prt] = 1.0
    return wst, sel, selbc


_NC_CACHE = {}


def kernel(x, w_compress, gamma, beta):
    x = np.ascontiguousarray(np.asarray(x, dtype=np.float32))
    w_compress = np.asarray(w_compress, dtype=np.float32)
    gamma = np.asarray(gamma, dtype=np.float32)
    beta = np.asarray(beta, dtype=np.float32)

    if "nc" not in _NC_CACHE:
        _NC_CACHE["nc"] = build_module()
    nc = _NC_CACHE["nc"]

    wst, sel, selbc = _host_constants(w_compress)
    in_maps = []
    for b in range(B):
        # xq[q, ch, blk, c] = x[b][ch, 4*blk+q, c]
        xq = np.ascontiguousarray(
            x[b].reshape(C, NB, 4, W).transpose(2, 0, 1, 3)
        )
        in_maps.append(
            {
                "xq": xq,
                "wst": wst,
                "sel": sel,
                "selbc": selbc,
                "gamma": gamma.reshape(4, 1),
                "beta": beta.reshape(4, 1),
            }
        )
    res = run_bass_kernel_spmd(
        nc,
        in_maps,
        core_ids=list(range(N_CORES)),
        trace=os.environ.get("BASSK_TRACE", "0") == "1",
    )
    _NC_CACHE["last_result"] = res
    # y[rr, o, t, c] -> out[o, 32t+rr, c]
    out = np.stack(
        [
            res.results[b]["y"].transpose(1, 2, 0, 3).reshape(4, H, W)
            for b in range(B)
        ],
        axis=0,
    )
    return out
